# revision 1
# baseline (speedup 1.0000x reference)
"""BertSelfAttention on 8 Trainium2 NeuronCores.

Sharding: data parallel over batch (B=2) x tensor parallel over heads
(16 heads -> 4 groups of 4). Core c handles batch c//4, heads 4*(c%4)..+4.
No collectives needed: each core produces a disjoint [256, 2048] slice of
the output (feature-major) which the host transposes/concatenates.

Per-core device program (identical on all cores, SPMD over data):
  inputs (host-prepped):
    xt    [1024, 2048]  hidden_states[b].T          (f32r)
    wq/wk/wv [1024, 256] weight column slices (wq,qb pre-scaled by 1/8)
    qb2/kb2 [128, 2]    bias chunks (per-partition layout)
    vb    [1, 256]
    maskc [128, 16]     additive mask chunks (mask[c*128+p] at [p, c])
  output:
    out   [256, 2048]   context slice, feature-major (host transposes)

  Stage A (projections, PE, f32r single-pass matmuls):
    Q.T, K.T feature-major  [128 feats(2 heads), 2048 tokens]
    V token-major [128 tokens x 16 tiles, 4*(64+ones col)]  (bf16)
  Stage B (attention per head):
    S_T[k, q] = K_h-tile.T @ Q_h   (PE, f32r, contraction over d=64)
    expS = exp(S_T + mask_k)       (ACT, mask as per-partition bias, ->bf16)
    ctxT/denom = V_aug-tile.T @ expS summed over k  (PE bf16, M=65:
                 rows 0-63 unnormalized ctx.T, row 64 softmax denom)
    normalize: DVE reciprocal of denom row + DMA partition-broadcast +
    DVE multiply; result stays feature-major [64, 2048] per head.
"""

import numpy as np

HIDDEN = 1024
HEADS = 16
HD = 64
B = 2
S = 2048
NCORES = 8
HPC = HEADS // 4  # heads per core = 4
WCOLS = HPC * HD  # 256 weight columns per core

_CACHE = {}


def _build_program():
    import concourse.bass as bass
    import concourse.bacc as bacc
    import concourse.tile as tile
    import concourse.mybir as mybir

    f32 = mybir.dt.float32
    f32r = mybir.dt.float32r
    bf16 = mybir.dt.bfloat16

    nc = bacc.Bacc("TRN2", target_bir_lowering=False, debug=False, num_devices=NCORES)

    xt_d = nc.dram_tensor("xt", [HIDDEN, S], f32r, kind="ExternalInput")
    wq_d = nc.dram_tensor("wq", [HIDDEN, WCOLS], f32r, kind="ExternalInput")
    wk_d = nc.dram_tensor("wk", [HIDDEN, WCOLS], f32r, kind="ExternalInput")
    wv_d = nc.dram_tensor("wv", [HIDDEN, WCOLS], f32r, kind="ExternalInput")
    qb_d = nc.dram_tensor("qb2", [128, 2], f32, kind="ExternalInput")
    kb_d = nc.dram_tensor("kb2", [128, 2], f32, kind="ExternalInput")
    vb_d = nc.dram_tensor("vb", [1, WCOLS], f32, kind="ExternalInput")
    mask_d = nc.dram_tensor("maskc", [128, 16], f32, kind="ExternalInput")
    out_d = nc.dram_tensor("out", [WCOLS, S], f32, kind="ExternalOutput")

    xt_r = xt_d.ap().rearrange("(c p) m -> p c m", p=128)  # [128, 8, 2048]
    wq_r = wq_d.ap().rearrange("(c p) n -> p c n", p=128)  # [128, 8, 256]
    wk_r = wk_d.ap().rearrange("(c p) n -> p c n", p=128)
    wv_r = wv_d.ap().rearrange("(c p) n -> p c n", p=128)

    with tile.TileContext(nc) as tc:
        with tc.tile_pool(name="persist", bufs=1) as persist:
            # persistent SBUF
            q_sb = persist.tile([128, 2, S], f32r)  # [feat(2 heads), mc, token]
            k_sb = persist.tile([128, 2, S], f32r)
            v_sb = persist.tile([128, 16, 4 * 65], f32r)  # [token, tile, 4*(64+one)]
            qkb = persist.tile([128, 20], f32)
            qb_sb = qkb[:, 0:2]
            kb_sb = qkb[:, 2:4]
            mask_sb = qkb[:, 4:20]
            v_blk = v_sb.rearrange("p m (l c) -> p m l c", l=4)
            vst = persist.tile([128, 32, 4], f32)
            nc.vector.memset(vst[:], 1.0)
            nc.vector.tensor_copy(v_blk[:, :, :, 0], vst[:, 0:16, :])
            ones_sb = vst.rearrange("p a b -> p (a b)")[0:1, 0:128]

            with (
                tc.tile_pool(name="wkp", bufs=1) as wkp,
            ):
              with (
                tc.tile_pool(name="proj", bufs=1) as proj,
                tc.tile_pool(name="ps_big", bufs=1, space="PSUM") as ps_big,
                tc.tile_pool(name="ps_sm", bufs=1, space="PSUM") as ps_sm,
              ):
                  xt = [proj.tile([128, S], f32r, tag=f"xt{k}", name=f"xt{k}") for k in range(8)]
                  wq_sb = proj.tile([128, 8, WCOLS], f32r)
                  wk_sb = proj.tile([128, 8, WCOLS], f32r)
                  # DMA order matters: wq first, then xt chunks, so Q-proj can
                  # start as soon as chunk 0 lands.
                  nc.sync.dma_start(out=wq_sb[:], in_=wq_r)
                  for k in range(8):
                      nc.sync.dma_start(out=xt[k][:], in_=xt_r[:, k, :])
                  nc.sync.dma_start(out=wk_sb[:], in_=wk_r)
                  nc.sync.dma_start(out=qb_sb, in_=qb_d.ap())
                  nc.sync.dma_start(out=kb_sb, in_=kb_d.ap())
                  nc.sync.dma_start(out=mask_sb, in_=mask_d.ap())

                  def proj_group(w_sb, b_sb, dst, mc, sp):
                      pq = ps_big.tile([128, 512], f32, tag="pq")
                      for k in range(8):
                          nc.tensor.matmul(
                              pq[:],
                              lhsT=w_sb[:, k, mc * 128 : mc * 128 + 128],
                              rhs=xt[k][:, sp * 512 : sp * 512 + 512],
                              start=(k == 0),
                              stop=(k == 7),
                          )
                      nc.vector.tensor_scalar_add(
                          dst[:, mc, sp * 512 : sp * 512 + 512],
                          pq[:],
                          b_sb[:, mc : mc + 1],
                      )

                  def v_group(mt, wv_sb=None, vb_sb=None):
                      pv = ps_big.tile([128, 512], f32, tag="pq", name="pv")[:, 0:256]
                      for k in range(8):
                          nc.tensor.matmul(
                              pv[:],
                              lhsT=xt[k][:, mt * 128 : mt * 128 + 128],
                              rhs=wv_sb[:, k, :],
                              start=(k == 0),
                              stop=False,
                          )
                      nc.tensor.matmul(
                          pv[:],
                          lhsT=ones_sb,
                          rhs=vb_sb[0:1, :],
                          start=False,
                          stop=True,
                      )
                      for lh in range(4):
                          nc.vector.tensor_copy(
                              v_sb[:, mt, 65 * lh + 1 : 65 * lh + 65],
                              pv[:, 64 * lh : 64 * lh + 64],
                          )

                  def s_pair(mc, sp, expP, kt, pool=None):
                      qs = sp * 512
                      ps = (pool or ps_big).tile([128, 1024], f32, tag="ps", bufs=2)
                      for half in range(2):
                          rs = 64 * half
                          nc.tensor.matmul(
                              ps[:, half * 512 : half * 512 + 512],
                              lhsT=k_sb[rs : rs + 64, mc, kt * 128 : kt * 128 + 128],
                              rhs=q_sb[rs : rs + 64, mc, qs : qs + 512],
                              start=True,
                              stop=True,
                          )
                      nc.scalar.activation(
                          expP[:, kt, :],
                          ps[:],
                          mybir.ActivationFunctionType.Exp,
                          bias=mask_sb[:, kt : kt + 1],
                      )

                  def ctx_head(mc, sp, expP, half, pool=None, pc_bufs=2):
                      lh = 2 * mc + half
                      qs = sp * 512
                      pc = (pool or ps_sm).tile([65, 512], f32, tag="pc", bufs=pc_bufs)
                      for kt in range(16):
                          nc.tensor.matmul(
                              pc[:],
                              lhsT=v_sb[:, kt, 65 * lh : 65 * lh + 65],
                              rhs=expP[:, kt, half * 512 : half * 512 + 512],
                              start=(kt == 0),
                              stop=(kt == 15),
                          )
                      ctxs = wkp.tile([65, 512], f32, tag="ctxs", bufs=2)
                      nc.vector.reciprocal(ctxs[0:1, :], pc[0:1, :])
                      bc = wkp.tile([65, 512], f32, tag="bc")
                      nc.gpsimd.partition_broadcast(bc[:], ctxs[0:1, :])
                      nc.vector.tensor_mul(ctxs[:], pc[:], bc[:])
                      nc.sync.dma_start(
                          out=out_d.ap()[64 * lh : 64 * lh + 64, qs : qs + 512],
                          in_=ctxs[1:65, :],
                      )

                  # ---- Stage A for pair 0, V for all heads ----
                  for sp in range(4):
                      proj_group(wq_sb, qb_sb, q_sb, 0, sp)
                  for sp in range(4):
                      proj_group(wk_sb, kb_sb, k_sb, 0, sp)
                  # ---- attention(pair 0) interleaved with V-proj + pair-1
                  # projections (PE filler while ACT chews the exps; Tile's
                  # slice-level deps let ctx(0,sp) start as V tiles land)
                  with (
                      tc.tile_pool(name="wvp", bufs=1) as wvp,
                      tc.tile_pool(name="att1", bufs=1) as att1,
                  ):
                      wv_sb = wvp.tile([128, 8, WCOLS], f32r)
                      vb_sb = wvp.tile([1, WCOLS], f32)
                      nc.sync.dma_start(out=wv_sb[:], in_=wv_r)
                      nc.sync.dma_start(out=vb_sb[:], in_=vb_d.ap())
                      for mt in range(16):
                          v_group(mt, wv_sb, vb_sb)
                      for sp in range(4):
                          expP = att1.tile([128, 16, 1024], f32r, tag="expP")
                          for kt in range(16):
                              s_pair(0, sp, expP, kt)
                          proj_group(wq_sb, qb_sb, q_sb, 1, sp)
                          proj_group(wk_sb, kb_sb, k_sb, 1, sp)
                          ctx_head(0, sp, expP, 0, pc_bufs=3)
                          ctx_head(0, sp, expP, 1, pc_bufs=3)

              # proj pool (xt + weights) is closed here; reuse the space for a
              # double-buffered expP so pair-1 S(sp+1) overlaps ctx(sp).
              with (
                  tc.tile_pool(name="att2", bufs=2) as att2,
                  tc.tile_pool(name="ps_big2", bufs=1, space="PSUM") as ps_big2,
                  tc.tile_pool(name="ps_sm2", bufs=1, space="PSUM") as ps_sm2,
              ):
                  expPs = {}
                  for sp in range(4):
                      expP = att2.tile([128, 16, 1024], f32r, tag="expP2")
                      expPs[sp] = expP
                      for kt in range(16):
                          s_pair(1, sp, expP, kt, pool=ps_big2)
                      if sp > 0:
                          ctx_head(1, sp - 1, expPs[sp - 1], 0, pool=ps_sm2, pc_bufs=4)
                          ctx_head(1, sp - 1, expPs[sp - 1], 1, pool=ps_sm2, pc_bufs=4)
                  ctx_head(1, 3, expPs[3], 0, pool=ps_sm2, pc_bufs=4)
                  ctx_head(1, 3, expPs[3], 1, pool=ps_sm2, pc_bufs=4)

    nc.compile()
    return nc


def _get_program():
    if "nc" not in _CACHE:
        _CACHE["nc"] = _build_program()
    return _CACHE["nc"]


def _make_in_maps(hidden_states, attention_mask, q_w, q_b, k_w, k_b, v_w, v_b):
    hs = np.asarray(hidden_states, np.float32)
    am = np.asarray(attention_mask, np.float32)
    q_w = np.asarray(q_w, np.float32)
    k_w = np.asarray(k_w, np.float32)
    v_w = np.asarray(v_w, np.float32)
    q_b = np.asarray(q_b, np.float32)
    k_b = np.asarray(k_b, np.float32)
    v_b = np.asarray(v_b, np.float32)

    scale = np.float32(1.0 / np.sqrt(HD))

    in_maps = []
    for c in range(NCORES):
        b = c // 4
        hg = c % 4
        cols = slice(WCOLS * hg, WCOLS * hg + WCOLS)
        mask = am[b, 0, 0, :]  # [S]
        in_maps.append(
            {
                "xt": np.ascontiguousarray(hs[b].T),
                "wq": np.ascontiguousarray(q_w[:, cols] * scale),
                "wk": np.ascontiguousarray(k_w[:, cols]),
                "wv": np.ascontiguousarray(v_w[:, cols]),
                "qb2": np.ascontiguousarray((q_b[cols] * scale).reshape(2, 128).T),
                "kb2": np.ascontiguousarray(k_b[cols].reshape(2, 128).T),
                "vb": np.ascontiguousarray(v_b[cols].reshape(1, WCOLS)),
                "maskc": np.ascontiguousarray(mask.reshape(16, 128).T),
            }
        )
    return in_maps


def kernel(hidden_states, attention_mask, q_w, q_b, k_w, k_b, v_w, v_b):
    from concourse import bass_utils

    nc = _get_program()
    in_maps = _make_in_maps(
        hidden_states, attention_mask, q_w, q_b, k_w, k_b, v_w, v_b
    )
    res = bass_utils.run_bass_kernel_spmd(nc, in_maps, core_ids=list(range(NCORES)))

    full = np.empty((B, S, HIDDEN), np.float32)
    for c in range(NCORES):
        b = c // 4
        hg = c % 4
        full[b, :, WCOLS * hg : WCOLS * hg + WCOLS] = res.results[c]["out"].T
    return full



# revision 3
# speedup vs baseline: 1.1867x; 1.1867x over previous
"""BertSelfAttention on 8 Trainium2 NeuronCores.

Sharding: data parallel over batch (B=2) x tensor parallel over heads
(16 heads -> 4 groups of 4). Core c handles batch c//4, heads 4*(c%4)..+4.
No collectives: each core produces a disjoint [256, 2048] slice of the
output (feature-major); the host transposes/concatenates.

v2 design (vs v1 baseline at 286us):
  The kernel is jointly limited by PE (~135us of matmul) and ACT
  (~143us of exp at 1 elem/cycle/lane); everything else must overlap.
  - software-pipeline over 8 "units" (head-pair mc x q-block sp):
    S-matmuls of unit u feed ACT(exp) for unit u while PE runs ctx of
    unit u-1 plus leftover projections as filler -> ACT stays dense.
  - inputs (xt + weights) DMA'd in bf16: halves the 8MB xt transfer
    that gated kernel start.
  - warmup matmuls on dummy data during the input DMA so the PE HAM
    clock-gate is released (1.2->2.4GHz) before real work arrives.
  - softmax denominator reciprocal via reciprocal_approx_fast (~5x
    faster than nc.vector.reciprocal; 51 ULP is plenty for a softmax
    denom) and only broadcast to 64 partitions after recip.
  - V bias folded into the PSUM->SBUF eviction (tensor_add with a
    broadcast bias tile) instead of a ones-row matmul.
  - expP/V in bf16 (PE rate identical, halves SBUF so two expP units
    stay in flight next to all projection state).

Per-core device program:
  inputs:
    xt    [1024, 2048] bf16  hidden_states[b].T
    wq/wk/wv [1024, 256] bf16 weight column slices (wq,qb pre-scaled 1/8)
    qb2/kb2 [128, 2] f32     bias chunks (per-partition layout)
    vb    [1, 256] f32
    maskc [128, 16] f32      additive mask chunks (mask[kt*128+p] at [p, kt])
  output:
    out   [256, 2048] f32    context slice, feature-major

  Stage A (projections, PE, bf16):
    Q.T, K.T feature-major [128 feats (2 heads), 2048 tokens] f32r
    V token-major [128 tok x 16 tiles, 4*(64+ones col)] bf16
  Stage B (attention per head-pair unit, pipelined):
    S_T[k, q] = K_h.T @ Q_h   (PE f32r, both heads packed via row groups)
    expS = exp(S_T + mask_k)  (ACT, mask as per-partition bias, ->bf16)
    ctxT/denom = V_aug.T @ expS summed over kt (PE bf16, M=65)
    normalize: recip_approx(denom) + gpsimd bcast + DVE mul, DMA out.
"""

import numpy as np

HIDDEN = 1024
HEADS = 16
HD = 64
B = 2
S = 2048
NCORES = 8
HPC = HEADS // 4  # heads per core = 4
WCOLS = HPC * HD  # 256 weight columns per core
WARM_MM = 26

_CACHE = {}


def _build_program():
    import concourse.bass as bass
    import concourse.bacc as bacc
    import concourse.tile as tile
    import concourse.mybir as mybir

    f32 = mybir.dt.float32
    f32r = mybir.dt.float32r
    bf16 = mybir.dt.bfloat16

    nc = bacc.Bacc("TRN2", target_bir_lowering=False, debug=False, num_devices=NCORES)

    xt_d = nc.dram_tensor("xt", [HIDDEN, S], bf16, kind="ExternalInput")
    wq_d = nc.dram_tensor("wq", [HIDDEN, WCOLS], bf16, kind="ExternalInput")
    wk_d = nc.dram_tensor("wk", [HIDDEN, WCOLS], bf16, kind="ExternalInput")
    wv_d = nc.dram_tensor("wv", [HIDDEN, WCOLS], bf16, kind="ExternalInput")
    qb_d = nc.dram_tensor("qb2", [128, 2], f32, kind="ExternalInput")
    kb_d = nc.dram_tensor("kb2", [128, 2], f32, kind="ExternalInput")
    vb_d = nc.dram_tensor("vb", [1, WCOLS], f32, kind="ExternalInput")
    mask_d = nc.dram_tensor("maskc", [128, 16], f32, kind="ExternalInput")
    out_d = nc.dram_tensor("out", [WCOLS, S], f32, kind="ExternalOutput")

    xt_r = xt_d.ap().rearrange("(c p) m -> p c m", p=128)  # [128, 8, 2048]
    wq_r = wq_d.ap().rearrange("(c p) n -> p c n", p=128)  # [128, 8, 256]
    wk_r = wk_d.ap().rearrange("(c p) n -> p c n", p=128)
    wv_r = wv_d.ap().rearrange("(c p) n -> p c n", p=128)

    with tile.TileContext(nc) as tc:
        with (
            tc.tile_pool(name="persist", bufs=1) as persist,
            tc.tile_pool(name="proj", bufs=1) as proj,
            tc.tile_pool(name="expp", bufs=2) as expp,
            tc.tile_pool(name="work", bufs=2) as work,
            tc.tile_pool(name="psp", bufs=1, space="PSUM") as psp,
        ):
            # ---- persistent SBUF ----
            q_sb = persist.tile([128, 2, S], f32r)  # [feat(2 heads), mc, token]
            k_sb = persist.tile([128, 2, S], f32r)
            v_sb = persist.tile([128, 16, 4 * 65], bf16)  # [tok, mt, 4*(1+64)]
            v_blk = v_sb.rearrange("p m (l c) -> p m l c", l=4)
            qkb = persist.tile([128, 20], f32)
            qb_sb = qkb[:, 0:2]
            kb_sb = qkb[:, 2:4]
            mask_sb = qkb[:, 4:20]
            vb_sb = persist.tile([1, WCOLS], f32)
            vbb = persist.tile([128, WCOLS], f32)
            vbb4 = vbb.rearrange("p (l c) -> p l c", l=4)
            wu = persist.tile([128, 512], bf16)

            # ---- input SBUF (weights + hidden states) ----
            xt = [proj.tile([128, S], bf16, tag=f"xt{k}", name=f"xt{k}") for k in range(8)]
            wq_sb = proj.tile([128, 8, WCOLS], bf16)
            wk_sb = proj.tile([128, 8, WCOLS], bf16)
            wv_sb = proj.tile([128, 8, WCOLS], bf16)

            # small inputs first (needed early, cheap), then K weights (first
            # projection), then xt chunks, then Q/V weights.
            nc.sync.dma_start(out=qb_sb, in_=qb_d.ap())
            nc.sync.dma_start(out=kb_sb, in_=kb_d.ap())
            nc.sync.dma_start(out=mask_sb, in_=mask_d.ap())
            nc.sync.dma_start(out=vb_sb[:], in_=vb_d.ap())
            nc.sync.dma_start(out=wk_sb[:], in_=wk_r)
            for k in range(8):
                nc.sync.dma_start(out=xt[k][:], in_=xt_r[:, k, :])
            nc.sync.dma_start(out=wq_sb[:], in_=wq_r)
            nc.sync.dma_start(out=wv_sb[:], in_=wv_r)

            # ---- PE warmup during the input DMA (HAM clock-gate release) ----
            nc.vector.memset(wu[:], 0.0)
            warm_ps = psp.tile([128, 1024], f32, tag="ps", bufs=2)
            for i in range(WARM_MM):
                nc.tensor.matmul(
                    warm_ps[:, 0:512],
                    lhsT=wu[:, 0:128],
                    rhs=wu[:],
                    start=True,
                    stop=True,
                )

            # ones column of V-aug (softmax denominator row source)
            nc.vector.memset(v_blk[:, :, :, 0:1], 1.0)
            # V bias broadcast to all partitions (folded into PSUM eviction)
            nc.gpsimd.partition_broadcast(vbb[:], vb_sb[0:1, :])

            # ---- building blocks ----
            def proj_group(w_sb, b_sb, dst, mc, sp):
                pq = psp.tile([128, 512], f32, tag="pq", bufs=2)
                for k in range(8):
                    nc.tensor.matmul(
                        pq[:],
                        lhsT=w_sb[:, k, mc * 128 : mc * 128 + 128],
                        rhs=xt[k][:, sp * 512 : sp * 512 + 512],
                        start=(k == 0),
                        stop=(k == 7),
                    )
                nc.vector.tensor_scalar_add(
                    dst[:, mc, sp * 512 : sp * 512 + 512],
                    pq[:],
                    b_sb[:, mc : mc + 1],
                )

            def v_group(mt):
                pv = psp.tile([128, WCOLS], f32, tag="pcv", bufs=2, name=f"pv{mt}")
                for k in range(8):
                    nc.tensor.matmul(
                        pv[:],
                        lhsT=xt[k][:, mt * 128 : mt * 128 + 128],
                        rhs=wv_sb[:, k, :],
                        start=(k == 0),
                        stop=(k == 7),
                    )
                pv4 = pv.rearrange("p (l c) -> p l c", l=4)
                nc.vector.tensor_add(v_blk[:, mt, :, 1:65], pv4[:], vbb4[:])

            def s_unit(mc, sp, kt, expP):
                qs = sp * 512
                ps = psp.tile([128, 1024], f32, tag="ps", bufs=2)
                for half in range(2):
                    rs = 64 * half
                    nc.tensor.matmul(
                        ps[:, half * 512 : half * 512 + 512],
                        lhsT=k_sb[rs : rs + 64, mc, kt * 128 : kt * 128 + 128],
                        rhs=q_sb[rs : rs + 64, mc, qs : qs + 512],
                        start=True,
                        stop=True,
                    )
                nc.scalar.activation(
                    expP[:, kt, :],
                    ps[:],
                    mybir.ActivationFunctionType.Exp,
                    bias=mask_sb[:, kt : kt + 1],
                )

            def ctx_quanta(mc, sp, expP, half):
                """Returns 5 callables: 4 matmul chunks + normalize/store."""
                lh = 2 * mc + half
                qs = sp * 512
                pc = psp.tile([65, 512], f32, tag="pcv", bufs=2, name=f"pc{lh}{sp}")

                def chunk(ci):
                    def go():
                        for kt in range(4 * ci, 4 * ci + 4):
                            nc.tensor.matmul(
                                pc[:],
                                lhsT=v_sb[:, kt, 65 * lh : 65 * lh + 65],
                                rhs=expP[:, kt, half * 512 : half * 512 + 512],
                                start=(kt == 0),
                                stop=(kt == 15),
                            )
                    return go

                def finish():
                    r = work.tile([1, 512], f32, tag="r")
                    nc.vector.reciprocal_approx_fast(r[:], pc[0:1, :])
                    bc = work.tile([65, 512], f32, tag="bc")
                    nc.gpsimd.partition_broadcast(bc[:], r[0:1, :])
                    ctxs = work.tile([65, 512], f32, tag="ctxs")
                    nc.vector.tensor_mul(ctxs[:], pc[:], bc[:])
                    nc.sync.dma_start(
                        out=out_d.ap()[64 * lh : 64 * lh + 64, qs : qs + 512],
                        in_=ctxs[1:65, :],
                    )

                return [chunk(0), chunk(1), chunk(2), chunk(3), finish]

            # ---- prelude projections: K(0, all sp) then Q(0, 0) ----
            for sp in range(4):
                proj_group(wk_sb, kb_sb, k_sb, 0, sp)
            proj_group(wq_sb, qb_sb, q_sb, 0, 0)

            # ---- software pipeline over 8 units ----
            UNITS = [(mc, sp) for mc in range(2) for sp in range(4)]
            expPs = {}

            def filler_for_step(u):
                f = []
                if u == 0:
                    for mt in range(16):
                        f.append(lambda mt=mt: v_group(mt))
                    f.append(lambda: proj_group(wq_sb, qb_sb, q_sb, 0, 1))
                    return f
                # ctx of the previous unit
                pmc, psp_ = UNITS[u - 1]
                pexp = expPs[u - 1]
                f += ctx_quanta(pmc, psp_, pexp, 0)
                f += ctx_quanta(pmc, psp_, pexp, 1)
                # remaining projections, each finishing a step before use
                if u == 1:
                    f.append(lambda: proj_group(wk_sb, kb_sb, k_sb, 1, 0))
                    f.append(lambda: proj_group(wk_sb, kb_sb, k_sb, 1, 1))
                    f.append(lambda: proj_group(wq_sb, qb_sb, q_sb, 0, 2))
                elif u == 2:
                    f.append(lambda: proj_group(wk_sb, kb_sb, k_sb, 1, 2))
                    f.append(lambda: proj_group(wk_sb, kb_sb, k_sb, 1, 3))
                    f.append(lambda: proj_group(wq_sb, qb_sb, q_sb, 0, 3))
                elif u == 3:
                    f.append(lambda: proj_group(wq_sb, qb_sb, q_sb, 1, 0))
                elif u == 4:
                    f.append(lambda: proj_group(wq_sb, qb_sb, q_sb, 1, 1))
                elif u == 5:
                    f.append(lambda: proj_group(wq_sb, qb_sb, q_sb, 1, 2))
                elif u == 6:
                    f.append(lambda: proj_group(wq_sb, qb_sb, q_sb, 1, 3))
                return f

            for u, (mc, sp) in enumerate(UNITS):
                expP = expp.tile([128, 16, 1024], bf16, tag="expP")
                expPs[u] = expP
                fillers = filler_for_step(u)
                nf = len(fillers)
                emitted = 0
                for kt in range(16):
                    s_unit(mc, sp, kt, expP)
                    want = (kt + 1) * nf // 16
                    while emitted < want:
                        fillers[emitted]()
                        emitted += 1

            # tail: ctx of the last unit
            pmc, psp_ = UNITS[7]
            for q in ctx_quanta(pmc, psp_, expPs[7], 0):
                q()
            for q in ctx_quanta(pmc, psp_, expPs[7], 1):
                q()

    nc.compile()
    return nc


def _get_program():
    if "nc" not in _CACHE:
        _CACHE["nc"] = _build_program()
    return _CACHE["nc"]


def _make_in_maps(hidden_states, attention_mask, q_w, q_b, k_w, k_b, v_w, v_b):
    import ml_dtypes

    bf16 = ml_dtypes.bfloat16

    hs = np.asarray(hidden_states, np.float32)
    am = np.asarray(attention_mask, np.float32)
    q_w = np.asarray(q_w, np.float32)
    k_w = np.asarray(k_w, np.float32)
    v_w = np.asarray(v_w, np.float32)
    q_b = np.asarray(q_b, np.float32)
    k_b = np.asarray(k_b, np.float32)
    v_b = np.asarray(v_b, np.float32)

    scale = np.float32(1.0 / np.sqrt(HD))

    xt_b = [np.ascontiguousarray(hs[b].T).astype(bf16) for b in range(B)]

    in_maps = []
    for c in range(NCORES):
        b = c // 4
        hg = c % 4
        cols = slice(WCOLS * hg, WCOLS * hg + WCOLS)
        mask = am[b, 0, 0, :]  # [S]
        in_maps.append(
            {
                "xt": xt_b[b],
                "wq": np.ascontiguousarray(q_w[:, cols] * scale).astype(bf16),
                "wk": np.ascontiguousarray(k_w[:, cols]).astype(bf16),
                "wv": np.ascontiguousarray(v_w[:, cols]).astype(bf16),
                "qb2": np.ascontiguousarray((q_b[cols] * scale).reshape(2, 128).T),
                "kb2": np.ascontiguousarray(k_b[cols].reshape(2, 128).T),
                "vb": np.ascontiguousarray(v_b[cols].reshape(1, WCOLS)),
                "maskc": np.ascontiguousarray(mask.reshape(16, 128).T),
            }
        )
    return in_maps


def kernel(hidden_states, attention_mask, q_w, q_b, k_w, k_b, v_w, v_b):
    from concourse import bass_utils

    nc = _get_program()
    in_maps = _make_in_maps(
        hidden_states, attention_mask, q_w, q_b, k_w, k_b, v_w, v_b
    )
    _CACHE["in_maps"] = in_maps
    res = bass_utils.run_bass_kernel_spmd(nc, in_maps, core_ids=list(range(NCORES)))

    full = np.empty((B, S, HIDDEN), np.float32)
    for c in range(NCORES):
        b = c // 4
        hg = c % 4
        full[b, :, WCOLS * hg : WCOLS * hg + WCOLS] = res.results[c]["out"].T
    return full


# revision 7
# speedup vs baseline: 1.4329x; 1.2074x over previous
"""BertSelfAttention on 8 Trainium2 NeuronCores.

Sharding: data parallel over batch (B=2) x tensor parallel over heads
(16 heads -> 4 groups of 4). Core c handles batch c//4, heads 4*(c%4)..+4.
No collectives: each core produces a disjoint [256, 2048] slice of the
output (feature-major); the host transposes/concatenates.

v3 design (baseline 286us, v2 241us):
  Jointly limited by PE (~135us of matmul) and ACT (~143-171us of exp,
  1 elem/cycle/lane, ScalarE-only); the kernel is a software pipeline
  over 8 "units" (head-pair mc x q-block sp) that keeps ACT dense:
  - inputs bf16; xt streamed as 16 half-chunk DMAs ordered so that
    K/Q projections and the first S-matmuls chase the transfer.
  - prelude K/Q projections accumulate in pq/pc PSUM slots so the
    S-ring (tag ps) is free the moment the first unit starts.
  - per unit: 16 S-pair matmuls (both heads packed via PE row groups)
    feed 16 exp ACTIVATEs; ctx matmuls of the SAME unit trail the exps
    in-step (lag-0), so there is no unit-boundary stall and no tail.
  - warmup matmuls during the DMA release the PE HAM clock gate.
  - softmax denom: reciprocal_approx_fast + gpsimd partition broadcast.
  - V bias folded into the PSUM->SBUF eviction (tensor_add).
  - expP/V in fp16 (PE rate identical to bf16/f32r for N=512).

Per-core device program:
  inputs:
    xt    [1024, 2048] bf16  hidden_states[b].T
    wq/wk/wv [1024, 256] bf16 weight column slices (wq,qb pre-scaled 1/8)
    qb2/kb2 [128, 2] f32     bias chunks (per-partition layout)
    vb    [1, 256] f32
    maskc [128, 16] f32      additive mask chunks (mask[kt*128+p] at [p, kt])
  output:
    out   [256, 2048] f32    context slice, feature-major
"""

import numpy as np

HIDDEN = 1024
HEADS = 16
HD = 64
B = 2
S = 2048
NCORES = 8
HPC = HEADS // 4  # heads per core = 4
WCOLS = HPC * HD  # 256 weight columns per core
WARM_MM = 12

_CACHE = {}


def _build_program():
    import concourse.bass as bass
    import concourse.bacc as bacc
    import concourse.tile as tile
    import concourse.mybir as mybir

    f32 = mybir.dt.float32
    f32r = mybir.dt.float32r
    bf16 = mybir.dt.bfloat16
    fp16 = mybir.dt.float16

    nc = bacc.Bacc("TRN2", target_bir_lowering=False, debug=False, num_devices=NCORES)

    xt_d = nc.dram_tensor("xt", [HIDDEN, S], bf16, kind="ExternalInput")
    wq_d = nc.dram_tensor("wq", [HIDDEN, WCOLS], bf16, kind="ExternalInput")
    wk_d = nc.dram_tensor("wk", [HIDDEN, WCOLS], bf16, kind="ExternalInput")
    wv_d = nc.dram_tensor("wv", [HIDDEN, WCOLS], bf16, kind="ExternalInput")
    qb_d = nc.dram_tensor("qb2", [128, 2], f32, kind="ExternalInput")
    kb_d = nc.dram_tensor("kb2", [128, 2], f32, kind="ExternalInput")
    vb_d = nc.dram_tensor("vb", [1, WCOLS], f32, kind="ExternalInput")
    mask_d = nc.dram_tensor("maskc", [128, 16], f32, kind="ExternalInput")
    out_d = nc.dram_tensor("out", [WCOLS, S], f32, kind="ExternalOutput")

    xt_r = xt_d.ap().rearrange("(c p) m -> p c m", p=128)  # [128, 8, 2048]
    wq_r = wq_d.ap().rearrange("(c p) n -> p c n", p=128)  # [128, 8, 256]
    wk_r = wk_d.ap().rearrange("(c p) n -> p c n", p=128)
    wv_r = wv_d.ap().rearrange("(c p) n -> p c n", p=128)

    with tile.TileContext(nc) as tc:
        with (
            tc.tile_pool(name="persist", bufs=1) as persist,
            tc.tile_pool(name="proj", bufs=1) as proj,
            tc.tile_pool(name="expp", bufs=2) as expp,
            tc.tile_pool(name="work", bufs=2) as work,
            tc.tile_pool(name="psp", bufs=1, space="PSUM") as psp,
        ):
            # ---- persistent SBUF ----
            q_sb = persist.tile([128, 2, S], f32r)  # [feat(2 heads), mc, token]
            k_sb = persist.tile([128, 2, S], f32r)
            v_sb = persist.tile([128, 16, 4 * 65], fp16)  # [tok, mt, 4*(1+64)]
            v_blk = v_sb.rearrange("p m (l c) -> p m l c", l=4)
            qkb = persist.tile([128, 20], f32)
            qb_sb = qkb[:, 0:2]
            kb_sb = qkb[:, 2:4]
            mask_sb = qkb[:, 4:20]
            vb_sb = persist.tile([1, WCOLS], f32)
            vbb = persist.tile([128, WCOLS], f32)
            vbb4 = vbb.rearrange("p (l c) -> p l c", l=4)
            wu = persist.tile([128, 512], bf16)

            # ---- input SBUF (weights + hidden states) ----
            xt = [proj.tile([128, S], bf16, tag=f"xt{k}", name=f"xt{k}") for k in range(8)]
            wq_sb = proj.tile([128, 8, WCOLS], bf16)
            wk_sb = proj.tile([128, 8, WCOLS], bf16)
            wv_sb = proj.tile([128, 8, WCOLS], bf16)

            # small inputs on the (otherwise idle) ACT queue
            nc.scalar.dma_start(out=qb_sb, in_=qb_d.ap())
            nc.scalar.dma_start(out=kb_sb, in_=kb_d.ap())
            nc.scalar.dma_start(out=mask_sb, in_=mask_d.ap())
            nc.scalar.dma_start(out=vb_sb[:], in_=vb_d.ap())
            # big stream on sync, ordered so compute chases the transfer:
            # wk, xt-half0[0:4], wq, xt-half0[4:8], wv, xt-half1[0:8]
            H = S // 2
            nc.sync.dma_start(out=wk_sb[:], in_=wk_r)
            for k in range(4):
                nc.sync.dma_start(out=xt[k][:, 0:H], in_=xt_r[:, k, 0:H])
            nc.sync.dma_start(out=wq_sb[:], in_=wq_r)
            for k in range(4, 8):
                nc.sync.dma_start(out=xt[k][:, 0:H], in_=xt_r[:, k, 0:H])
            nc.sync.dma_start(out=wv_sb[:], in_=wv_r)
            for k in range(8):
                nc.sync.dma_start(out=xt[k][:, H:S], in_=xt_r[:, k, H:S])

            # ---- PE warmup during the input DMA (HAM clock-gate release) ----
            nc.vector.memset(wu[:], 0.0)
            warm_ps = psp.tile([128, 512], f32, tag="pq", bufs=2, name="warm")
            for i in range(WARM_MM):
                nc.tensor.matmul(
                    warm_ps[:, 0:256],
                    lhsT=wu[:, 0:128],
                    rhs=wu[:, 0:256],
                    start=True,
                    stop=True,
                )

            # ones column of V-aug (softmax denominator row source)
            nc.vector.memset(v_blk[:, :, :, 0:1], 1.0)
            # V bias broadcast to all partitions (folded into PSUM eviction)
            nc.gpsimd.partition_broadcast(vbb[:], vb_sb[0:1, :])

            # ---- building blocks ----
            def proj_group(w_sb, b_sb, dst, mc, sp, tag="pq"):
                pq = psp.tile([128, 512], f32, tag=tag, bufs=2)
                for k in range(8):
                    nc.tensor.matmul(
                        pq[:],
                        lhsT=w_sb[:, k, mc * 128 : mc * 128 + 128],
                        rhs=xt[k][:, sp * 512 : sp * 512 + 512],
                        start=(k == 0),
                        stop=(k == 7),
                    )
                nc.vector.tensor_scalar_add(
                    dst[:, mc, sp * 512 : sp * 512 + 512],
                    pq[:],
                    b_sb[:, mc : mc + 1],
                )

            def v_group(mt):
                pv = psp.tile([128, 512], f32, tag="pq", bufs=2, name=f"pv{mt}")
                for k in range(8):
                    nc.tensor.matmul(
                        pv[:, 0:WCOLS],
                        lhsT=xt[k][:, mt * 128 : mt * 128 + 128],
                        rhs=wv_sb[:, k, :],
                        start=(k == 0),
                        stop=(k == 7),
                    )
                pv4 = pv[:, 0:WCOLS].rearrange("p (l c) -> p l c", l=4)
                nc.vector.tensor_add(v_blk[:, mt, :, 1:65], pv4, vbb4[:])

            def s_unit(mc, sp, kt, expP):
                qs = sp * 512
                ps = psp.tile([128, 1024], f32, tag="ps", bufs=2)
                for half in range(2):
                    rs = 64 * half
                    nc.tensor.matmul(
                        ps[:, half * 512 : half * 512 + 512],
                        lhsT=k_sb[rs : rs + 64, mc, kt * 128 : kt * 128 + 128],
                        rhs=q_sb[rs : rs + 64, mc, qs : qs + 512],
                        start=True,
                        stop=True,
                    )
                nc.scalar.activation(
                    expP[:, kt, :],
                    ps[:],
                    mybir.ActivationFunctionType.Exp,
                    bias=mask_sb[:, kt : kt + 1],
                )

            def ctx_quanta(mc, sp, expP, half):
                """Returns 5 callables: 4 matmul chunks + normalize/store."""
                lh = 2 * mc + half
                qs = sp * 512
                pc = psp.tile([65, 512], f32, tag="pc", bufs=2, name=f"pc{lh}{sp}")

                def chunk(ci):
                    def go():
                        for kt in range(4 * ci, 4 * ci + 4):
                            nc.tensor.matmul(
                                pc[:],
                                lhsT=v_sb[:, kt, 65 * lh : 65 * lh + 65],
                                rhs=expP[:, kt, half * 512 : half * 512 + 512],
                                start=(kt == 0),
                                stop=(kt == 15),
                            )
                    return go

                def finish():
                    r = work.tile([1, 512], f32, tag="r")
                    nc.vector.reciprocal_approx_fast(r[:], pc[0:1, :])
                    bc = work.tile([65, 512], f32, tag="bc")
                    nc.gpsimd.partition_broadcast(bc[:], r[0:1, :])
                    ctxs = work.tile([65, 512], f32, tag="ctxs")
                    nc.vector.tensor_mul(ctxs[:], pc[:], bc[:])
                    nc.sync.dma_start(
                        out=out_d.ap()[64 * lh : 64 * lh + 64, qs : qs + 512],
                        in_=ctxs[1:65, :],
                    )

                return [chunk(0), chunk(1), chunk(2), chunk(3), finish]

            # ---- prelude: K(0,0), K(0,1), Q(0,0) chase the half0 stream ----
            # pq slots hold K(0,0)/K(0,1); a pc-tag slot holds Q(0,0) so the
            # S-ring (tag ps) stays free for the first unit.
            pre_k0 = psp.tile([128, 512], f32, tag="pq", bufs=2, name="preK0")
            pre_k1 = psp.tile([128, 512], f32, tag="pq", bufs=2, name="preK1")
            pre_q = psp.tile([128, 512], f32, tag="pc", bufs=2, name="preQ")
            for k in range(8):
                nc.tensor.matmul(
                    pre_k0[:],
                    lhsT=wk_sb[:, k, 0:128],
                    rhs=xt[k][:, 0:512],
                    start=(k == 0),
                    stop=(k == 7),
                )
                nc.tensor.matmul(
                    pre_k1[:],
                    lhsT=wk_sb[:, k, 0:128],
                    rhs=xt[k][:, 512:1024],
                    start=(k == 0),
                    stop=(k == 7),
                )
                nc.tensor.matmul(
                    pre_q[:],
                    lhsT=wq_sb[:, k, 0:128],
                    rhs=xt[k][:, 0:512],
                    start=(k == 0),
                    stop=(k == 7),
                )
            nc.vector.tensor_scalar_add(k_sb[:, 0, 0:512], pre_k0[:], kb_sb[:, 0:1])
            nc.vector.tensor_scalar_add(k_sb[:, 0, 512:1024], pre_k1[:], kb_sb[:, 0:1])
            nc.vector.tensor_scalar_add(q_sb[:, 0, 0:512], pre_q[:], qb_sb[:, 0:1])

            # ---- software pipeline over 8 units, ctx trails exps in-step ----
            UNITS = [(mc, sp) for mc in range(2) for sp in range(4)]
            expPs = {}
            ctx_q = {}  # u -> list of 10 quanta [h0c0..h0fin, h1c0..h1fin]

            def fillers_for_unit(u):
                """kt-slot -> list of callables, emitted after s_unit(u, kt)."""
                slots = {kt: [] for kt in range(16)}

                def put(kt, fn):
                    slots[kt].append(fn)

                if u == 0:
                    # K(0,2)/K(0,3) chase the half1 stream; V + Q(0,1) fill.
                    put(0, lambda: proj_group(wk_sb, kb_sb, k_sb, 0, 2))
                    put(1, lambda: proj_group(wk_sb, kb_sb, k_sb, 0, 3))
                    for mt in range(12):
                        put(2 + mt, lambda mt=mt: v_group(mt))
                    put(8, lambda: proj_group(wq_sb, qb_sb, q_sb, 0, 1))
                    for mt in range(12, 16):
                        put(mt - 2, lambda mt=mt: v_group(mt))
                    return slots

                # carried tail of the previous unit's in-step ctx
                if u >= 2:
                    pq_ = ctx_q[u - 1]
                    put(0, pq_[3])   # h0 c3
                    put(1, pq_[8])   # h1 c3
                    put(2, pq_[4])   # h0 finish
                    put(2, pq_[9])   # h1 finish
                if u == 1:
                    # lag-1 ctx of unit 0, one quantum per slot (ACT pace)
                    q0 = ctx_q[0] = ctx_quanta(0, 0, expPs[0], 0) + ctx_quanta(
                        0, 0, expPs[0], 1
                    )
                    for i in range(10):
                        put(i, q0[i])
                # in-step ctx of this unit
                mc, sp = UNITS[u]
                qu = ctx_q[u] = ctx_quanta(mc, sp, expPs[u], 0) + ctx_quanta(
                    mc, sp, expPs[u], 1
                )
                if u == 1:
                    put(6, qu[0]); put(7, qu[5])
                    put(10, qu[1]); put(11, qu[6])
                    put(13, qu[2]); put(14, qu[7])
                else:
                    put(4, qu[0])    # h0 c0 (needs exps kt0-3)
                    put(5, qu[5])    # h1 c0
                    put(8, qu[1])    # h0 c1
                    put(9, qu[6])    # h1 c1
                    put(12, qu[2])   # h0 c2
                    put(13, qu[7])   # h1 c2
                # h0/h1 c3 + finishes carried into unit u+1 (or the tail)

                # remaining projections, each a step before first use
                if u == 1:
                    put(12, lambda: proj_group(wq_sb, qb_sb, q_sb, 0, 2))
                elif u == 2:
                    put(6, lambda: proj_group(wk_sb, kb_sb, k_sb, 1, 0))
                    put(7, lambda: proj_group(wk_sb, kb_sb, k_sb, 1, 1))
                    put(10, lambda: proj_group(wq_sb, qb_sb, q_sb, 0, 3))
                elif u == 3:
                    put(6, lambda: proj_group(wk_sb, kb_sb, k_sb, 1, 2))
                    put(7, lambda: proj_group(wk_sb, kb_sb, k_sb, 1, 3))
                    put(10, lambda: proj_group(wq_sb, qb_sb, q_sb, 1, 0))
                elif u == 4:
                    put(6, lambda: proj_group(wq_sb, qb_sb, q_sb, 1, 1))
                elif u == 5:
                    put(6, lambda: proj_group(wq_sb, qb_sb, q_sb, 1, 2))
                elif u == 6:
                    put(6, lambda: proj_group(wq_sb, qb_sb, q_sb, 1, 3))
                return slots

            for u, (mc, sp) in enumerate(UNITS):
                expP = expp.tile([128, 16, 1024], fp16, tag="expP")
                expPs[u] = expP
                slots = fillers_for_unit(u)
                for kt in range(16):
                    s_unit(mc, sp, kt, expP)
                    for fn in slots[kt]:
                        fn()

            # tail: last unit's c3 + finishes
            q7 = ctx_q[7]
            q7[3](); q7[8](); q7[4](); q7[9]()

    nc.compile()
    return nc


def _get_program():
    if "nc" not in _CACHE:
        _CACHE["nc"] = _build_program()
    return _CACHE["nc"]


def _make_in_maps(hidden_states, attention_mask, q_w, q_b, k_w, k_b, v_w, v_b):
    import ml_dtypes

    bf16 = ml_dtypes.bfloat16

    hs = np.asarray(hidden_states, np.float32)
    am = np.asarray(attention_mask, np.float32)
    q_w = np.asarray(q_w, np.float32)
    k_w = np.asarray(k_w, np.float32)
    v_w = np.asarray(v_w, np.float32)
    q_b = np.asarray(q_b, np.float32)
    k_b = np.asarray(k_b, np.float32)
    v_b = np.asarray(v_b, np.float32)

    scale = np.float32(1.0 / np.sqrt(HD))

    xt_b = [np.ascontiguousarray(hs[b].T).astype(bf16) for b in range(B)]

    in_maps = []
    for c in range(NCORES):
        b = c // 4
        hg = c % 4
        cols = slice(WCOLS * hg, WCOLS * hg + WCOLS)
        mask = am[b, 0, 0, :]  # [S]
        in_maps.append(
            {
                "xt": xt_b[b],
                "wq": np.ascontiguousarray(q_w[:, cols] * scale).astype(bf16),
                "wk": np.ascontiguousarray(k_w[:, cols]).astype(bf16),
                "wv": np.ascontiguousarray(v_w[:, cols]).astype(bf16),
                "qb2": np.ascontiguousarray((q_b[cols] * scale).reshape(2, 128).T),
                "kb2": np.ascontiguousarray(k_b[cols].reshape(2, 128).T),
                "vb": np.ascontiguousarray(v_b[cols].reshape(1, WCOLS)),
                "maskc": np.ascontiguousarray(mask.reshape(16, 128).T),
            }
        )
    return in_maps


def kernel(hidden_states, attention_mask, q_w, q_b, k_w, k_b, v_w, v_b):
    from concourse import bass_utils

    nc = _get_program()
    in_maps = _make_in_maps(
        hidden_states, attention_mask, q_w, q_b, k_w, k_b, v_w, v_b
    )
    _CACHE["in_maps"] = in_maps
    res = bass_utils.run_bass_kernel_spmd(nc, in_maps, core_ids=list(range(NCORES)))

    full = np.empty((B, S, HIDDEN), np.float32)
    for c in range(NCORES):
        b = c // 4
        hg = c % 4
        full[b, :, WCOLS * hg : WCOLS * hg + WCOLS] = res.results[c]["out"].T
    return full


# revision 11
# speedup vs baseline: 1.4513x; 1.0129x over previous
"""BertSelfAttention on 8 Trainium2 NeuronCores.

Sharding: data parallel over batch (B=2) x tensor parallel over heads
(16 heads -> 4 groups of 4). Core c handles batch c//4, heads 4*(c%4)..+4.
No collectives: each core produces a disjoint [256, 2048] slice of the
output (feature-major); the host transposes/concatenates.

v3 design (baseline 286us, v2 241us):
  Jointly limited by PE (~135us of matmul) and ACT (~143-171us of exp,
  1 elem/cycle/lane, ScalarE-only); the kernel is a software pipeline
  over 8 "units" (head-pair mc x q-block sp) that keeps ACT dense:
  - inputs bf16; xt streamed as 16 half-chunk DMAs ordered so that
    K/Q projections and the first S-matmuls chase the transfer.
  - prelude K/Q projections accumulate in pq/pc PSUM slots so the
    S-ring (tag ps) is free the moment the first unit starts.
  - per unit: 16 S-pair matmuls (both heads packed via PE row groups)
    feed 16 exp ACTIVATEs; ctx matmuls of the SAME unit trail the exps
    in-step (lag-0), so there is no unit-boundary stall and no tail.
  - warmup matmuls during the DMA release the PE HAM clock gate.
  - softmax denom: reciprocal_approx_fast + gpsimd partition broadcast.
  - V bias folded into the PSUM->SBUF eviction (tensor_add).
  - expP/V in fp16 (PE rate identical to bf16/f32r for N=512).

Per-core device program:
  inputs:
    xt    [1024, 2048] bf16  hidden_states[b].T
    wq/wk/wv [1024, 256] bf16 weight column slices (wq,qb pre-scaled 1/8)
    qb2/kb2 [128, 2] f32     bias chunks (per-partition layout)
    vb    [1, 256] f32
    maskc [128, 16] f32      additive mask chunks (mask[kt*128+p] at [p, kt])
  output:
    out   [256, 2048] f32    context slice, feature-major
"""

import numpy as np

HIDDEN = 1024
HEADS = 16
HD = 64
B = 2
S = 2048
NCORES = 8
HPC = HEADS // 4  # heads per core = 4
WCOLS = HPC * HD  # 256 weight columns per core
WARM_MM = 12

_CACHE = {}


def _build_program():
    import concourse.bass as bass
    import concourse.bacc as bacc
    import concourse.tile as tile
    import concourse.mybir as mybir

    f32 = mybir.dt.float32
    f32r = mybir.dt.float32r
    bf16 = mybir.dt.bfloat16
    fp16 = mybir.dt.float16

    nc = bacc.Bacc("TRN2", target_bir_lowering=False, debug=False, num_devices=NCORES)

    xt_d = nc.dram_tensor("xt", [HIDDEN, S], bf16, kind="ExternalInput")
    wq_d = nc.dram_tensor("wq", [HIDDEN, WCOLS], bf16, kind="ExternalInput")
    wk_d = nc.dram_tensor("wk", [HIDDEN, WCOLS], bf16, kind="ExternalInput")
    wv_d = nc.dram_tensor("wv", [HIDDEN, WCOLS], bf16, kind="ExternalInput")
    qb_d = nc.dram_tensor("qb2", [128, 2], f32, kind="ExternalInput")
    kb_d = nc.dram_tensor("kb2", [128, 2], f32, kind="ExternalInput")
    vb_d = nc.dram_tensor("vb", [1, WCOLS], f32, kind="ExternalInput")
    mask_d = nc.dram_tensor("maskc", [128, 16], f32, kind="ExternalInput")
    out_d = nc.dram_tensor("out", [WCOLS, S], f32, kind="ExternalOutput")

    xt_r = xt_d.ap().rearrange("(c p) m -> p c m", p=128)  # [128, 8, 2048]
    wq_r = wq_d.ap().rearrange("(c p) n -> p c n", p=128)  # [128, 8, 256]
    wk_r = wk_d.ap().rearrange("(c p) n -> p c n", p=128)
    wv_r = wv_d.ap().rearrange("(c p) n -> p c n", p=128)

    with tile.TileContext(nc) as tc:
        with (
            tc.tile_pool(name="persist", bufs=1) as persist,
            tc.tile_pool(name="proj", bufs=1) as proj,
            tc.tile_pool(name="expp", bufs=2) as expp,
            tc.tile_pool(name="work", bufs=2) as work,
            tc.tile_pool(name="psp", bufs=1, space="PSUM") as psp,
        ):
            # ---- persistent SBUF ----
            q_sb = persist.tile([128, 2, S], f32r)  # [feat(2 heads), mc, token]
            k_sb = persist.tile([128, 2, S], f32r)
            v_sb = persist.tile([128, 16, 4 * 65], fp16)  # [tok, mt, 4*(1+64)]
            v_blk = v_sb.rearrange("p m (l c) -> p m l c", l=4)
            qkb = persist.tile([128, 20], f32)
            qb_sb = qkb[:, 0:2]
            kb_sb = qkb[:, 2:4]
            mask_sb = qkb[:, 4:20]
            vb_sb = persist.tile([1, WCOLS], f32)
            vbb = persist.tile([128, WCOLS], f32)
            vbb4 = vbb.rearrange("p (l c) -> p l c", l=4)
            wu = persist.tile([128, 512], bf16)

            # ---- input SBUF (weights + hidden states) ----
            xt = [proj.tile([128, S], bf16, tag=f"xt{k}", name=f"xt{k}") for k in range(8)]
            wq_sb = proj.tile([128, 8, WCOLS], bf16)
            wk_sb = proj.tile([128, 8, WCOLS], bf16)
            wv_sb = proj.tile([128, 8, WCOLS], bf16)

            # small inputs on the (otherwise idle) ACT queue
            nc.scalar.dma_start(out=qb_sb, in_=qb_d.ap())
            nc.scalar.dma_start(out=kb_sb, in_=kb_d.ap())
            nc.scalar.dma_start(out=mask_sb, in_=mask_d.ap())
            nc.scalar.dma_start(out=vb_sb[:], in_=vb_d.ap())
            # big stream on sync, ordered so compute chases the transfer:
            # only the mc=0 halves of wk/wq gate the prelude.
            H = S // 2
            nc.sync.dma_start(out=wk_sb[:, :, 0:128], in_=wk_r[:, :, 0:128])
            for k in range(8):
                nc.sync.dma_start(out=xt[k][:, 0:H], in_=xt_r[:, k, 0:H])
            nc.sync.dma_start(out=wq_sb[:, :, 0:128], in_=wq_r[:, :, 0:128])
            nc.sync.dma_start(out=wv_sb[:], in_=wv_r)
            for k in range(8):
                nc.sync.dma_start(out=xt[k][:, H:S], in_=xt_r[:, k, H:S])
            nc.sync.dma_start(out=wk_sb[:, :, 128:256], in_=wk_r[:, :, 128:256])
            nc.sync.dma_start(out=wq_sb[:, :, 128:256], in_=wq_r[:, :, 128:256])

            # ---- PE warmup during the input DMA (HAM clock-gate release) ----
            nc.vector.memset(wu[:], 0.0)
            warm_ps = psp.tile([128, 512], f32, tag="pq", bufs=2, name="warm")
            for i in range(WARM_MM):
                nc.tensor.matmul(
                    warm_ps[:, 0:256],
                    lhsT=wu[:, 0:128],
                    rhs=wu[:, 0:256],
                    start=True,
                    stop=True,
                )

            # ones column of V-aug (softmax denominator row source)
            nc.vector.memset(v_blk[:, :, :, 0:1], 1.0)
            # V bias broadcast to all partitions (folded into PSUM eviction)
            nc.gpsimd.partition_broadcast(vbb[:], vb_sb[0:1, :])

            # ---- building blocks ----
            def proj_group(w_sb, b_sb, dst, mc, sp, tag="pq"):
                pq = psp.tile([128, 512], f32, tag=tag, bufs=2)
                for k in range(8):
                    nc.tensor.matmul(
                        pq[:],
                        lhsT=w_sb[:, k, mc * 128 : mc * 128 + 128],
                        rhs=xt[k][:, sp * 512 : sp * 512 + 512],
                        start=(k == 0),
                        stop=(k == 7),
                    )
                nc.vector.tensor_scalar_add(
                    dst[:, mc, sp * 512 : sp * 512 + 512],
                    pq[:],
                    b_sb[:, mc : mc + 1],
                )

            def v_group(mt):
                pv = psp.tile([128, 512], f32, tag="pq", bufs=2, name=f"pv{mt}")
                for k in range(8):
                    nc.tensor.matmul(
                        pv[:, 0:WCOLS],
                        lhsT=xt[k][:, mt * 128 : mt * 128 + 128],
                        rhs=wv_sb[:, k, :],
                        start=(k == 0),
                        stop=(k == 7),
                    )
                pv4 = pv[:, 0:WCOLS].rearrange("p (l c) -> p l c", l=4)
                nc.vector.tensor_add(v_blk[:, mt, :, 1:65], pv4, vbb4[:])

            def s_unit(mc, sp, kt, expP):
                qs = sp * 512
                ps = psp.tile([128, 1024], f32, tag="ps", bufs=2)
                for half in range(2):
                    rs = 64 * half
                    nc.tensor.matmul(
                        ps[:, half * 512 : half * 512 + 512],
                        lhsT=k_sb[rs : rs + 64, mc, kt * 128 : kt * 128 + 128],
                        rhs=q_sb[rs : rs + 64, mc, qs : qs + 512],
                        start=True,
                        stop=True,
                    )
                nc.scalar.activation(
                    expP[:, kt, :],
                    ps[:],
                    mybir.ActivationFunctionType.Exp,
                    bias=mask_sb[:, kt : kt + 1],
                )

            def ctx_quanta(mc, sp, expP, half):
                """Returns 5 callables: 4 matmul chunks + normalize/store."""
                lh = 2 * mc + half
                qs = sp * 512
                pc = psp.tile([65, 512], f32, tag="pc", bufs=2, name=f"pc{lh}{sp}")

                def chunk(ci):
                    def go():
                        for kt in range(4 * ci, 4 * ci + 4):
                            nc.tensor.matmul(
                                pc[:],
                                lhsT=v_sb[:, kt, 65 * lh : 65 * lh + 65],
                                rhs=expP[:, kt, half * 512 : half * 512 + 512],
                                start=(kt == 0),
                                stop=(kt == 15),
                            )
                    return go

                def finish():
                    r = work.tile([1, 512], f32, tag="r")
                    nc.vector.reciprocal_approx_fast(r[:], pc[0:1, :])
                    bc = work.tile([65, 512], f32, tag="bc")
                    nc.gpsimd.partition_broadcast(bc[:], r[0:1, :])
                    ctxs = work.tile([65, 512], f32, tag="ctxs")
                    nc.vector.tensor_mul(ctxs[:], pc[:], bc[:])
                    nc.sync.dma_start(
                        out=out_d.ap()[64 * lh : 64 * lh + 64, qs : qs + 512],
                        in_=ctxs[1:65, :],
                    )

                return [chunk(0), chunk(1), chunk(2), chunk(3), finish]

            # ---- prelude: K(0,0), K(0,1), Q(0,0) chase the half0 stream ----
            # pq slots hold K(0,0)/K(0,1); a pc-tag slot holds Q(0,0) so the
            # S-ring (tag ps) stays free for the first unit.
            pre_k0 = psp.tile([128, 512], f32, tag="pq", bufs=2, name="preK0")
            pre_k1 = psp.tile([128, 512], f32, tag="pq", bufs=2, name="preK1")
            pre_q = psp.tile([128, 512], f32, tag="pc", bufs=2, name="preQ")
            for k in range(8):
                nc.tensor.matmul(
                    pre_k0[:],
                    lhsT=wk_sb[:, k, 0:128],
                    rhs=xt[k][:, 0:512],
                    start=(k == 0),
                    stop=(k == 7),
                )
                nc.tensor.matmul(
                    pre_k1[:],
                    lhsT=wk_sb[:, k, 0:128],
                    rhs=xt[k][:, 512:1024],
                    start=(k == 0),
                    stop=(k == 7),
                )
                nc.tensor.matmul(
                    pre_q[:],
                    lhsT=wq_sb[:, k, 0:128],
                    rhs=xt[k][:, 0:512],
                    start=(k == 0),
                    stop=(k == 7),
                )
            nc.vector.tensor_scalar_add(k_sb[:, 0, 0:512], pre_k0[:], kb_sb[:, 0:1])
            nc.vector.tensor_scalar_add(q_sb[:, 0, 0:512], pre_q[:], qb_sb[:, 0:1])
            nc.vector.tensor_scalar_add(k_sb[:, 0, 512:1024], pre_k1[:], kb_sb[:, 0:1])

            # ---- software pipeline over 8 units, ctx trails exps in-step ----
            UNITS = [(mc, sp) for mc in range(2) for sp in range(4)]
            expPs = {}
            ctx_q = {}  # u -> list of 10 quanta [h0c0..h0fin, h1c0..h1fin]

            def K(mc, sp):
                return lambda: proj_group(wk_sb, kb_sb, k_sb, mc, sp)

            def Q(mc, sp):
                return lambda: proj_group(wq_sb, qb_sb, q_sb, mc, sp)

            def fillers_for_unit(u):
                """kt-slot -> list of callables, emitted after s_unit(u, kt).

                All ctx is lag-1 (unit u runs ctx of unit u-1), except unit 7
                whose own ctx trails its exps in-step so there is no tail.
                Per-unit PE filler load is balanced against the ACT window
                (~17.8us/unit): V-projection is split across units 0-1,
                interleaved so ctx(0,0) chunk ci (which reads only V tiles
                4ci..4ci+3) finds its V tiles ready.
                """
                slots = {kt: [] for kt in range(16)}

                def put(kt, fn):
                    slots[kt].append(fn)

                def lag1(u):
                    # previous unit's ctx quanta, both heads interleaved
                    pmc, psp_ = UNITS[u - 1]
                    q = ctx_q[u - 1] = ctx_quanta(
                        pmc, psp_, expPs[u - 1], 0
                    ) + ctx_quanta(pmc, psp_, expPs[u - 1], 1)
                    return q

                if u == 0:
                    # K(0,2)/(0,3) chase the half1 stream (gate exps kt8/12)
                    put(0, K(0, 2))
                    put(1, K(0, 3))
                    for mt in range(8):
                        put(2 + mt, lambda mt=mt: v_group(mt))
                    put(11, Q(0, 1))
                    put(13, Q(0, 2))
                elif u == 1:
                    q = lag1(u)
                    put(0, lambda: v_group(8))
                    put(1, lambda: v_group(9))
                    put(2, q[0])                      # h0 c0
                    put(3, lambda: v_group(10))
                    put(4, q[1])                      # h0 c1
                    put(5, lambda: v_group(11))
                    put(6, q[5])                      # h1 c0
                    put(7, lambda: v_group(12))
                    put(8, q[6])                      # h1 c1
                    put(9, lambda: v_group(13))
                    put(10, q[2])                     # h0 c2
                    put(11, lambda: v_group(14))
                    put(12, q[7])                     # h1 c2
                    put(13, lambda: v_group(15))
                    put(14, q[3])                     # h0 c3
                    put(15, q[8])                     # h1 c3
                    put(15, q[4])                     # h0 finish
                    put(15, q[9])                     # h1 finish
                else:
                    q = lag1(u)
                    if u == 7:
                        # compact: free both pc slots by slot 4 so the
                        # in-step ctx(1,3) can allocate them from slot 5
                        for i, slot in zip(
                            (0, 5, 1, 6, 2, 7, 3, 8, 4, 9),
                            (0, 0, 1, 1, 2, 2, 3, 3, 4, 4),
                        ):
                            put(slot, q[i])
                    else:
                        for i, slot in zip(
                            (0, 5, 1, 6, 2, 7, 3, 8, 4, 9),
                            (0, 1, 2, 3, 4, 5, 6, 7, 8, 9),
                        ):
                            put(slot, q[i])
                    if u == 2:
                        put(10, Q(0, 3))
                        put(11, K(1, 0))
                    elif u == 3:
                        put(10, K(1, 1))
                        put(12, K(1, 2))
                        put(14, Q(1, 0))
                    elif u == 4:
                        put(0, K(1, 3))
                        put(12, Q(1, 1))
                    elif u == 5:
                        put(10, Q(1, 2))
                    elif u == 6:
                        put(10, Q(1, 3))
                    elif u == 7:
                        # own ctx trails in-step
                        mc, sp = UNITS[7]
                        qu = ctx_q[7] = ctx_quanta(
                            mc, sp, expPs[7], 0
                        ) + ctx_quanta(mc, sp, expPs[7], 1)
                        put(5, qu[0]); put(6, qu[5])
                        put(8, qu[1]); put(9, qu[6])
                        put(12, qu[2]); put(13, qu[7])
                return slots

            for u, (mc, sp) in enumerate(UNITS):
                expP = expp.tile([128, 16, 1024], fp16, tag="expP")
                expPs[u] = expP
                slots = fillers_for_unit(u)
                for kt in range(16):
                    s_unit(mc, sp, kt, expP)
                    for fn in slots[kt]:
                        fn()

            # tail: last unit's c3 + finishes
            q7 = ctx_q[7]
            q7[3](); q7[8](); q7[4](); q7[9]()

    nc.compile()
    return nc


def _get_program():
    if "nc" not in _CACHE:
        _CACHE["nc"] = _build_program()
    return _CACHE["nc"]


def _make_in_maps(hidden_states, attention_mask, q_w, q_b, k_w, k_b, v_w, v_b):
    import ml_dtypes

    bf16 = ml_dtypes.bfloat16

    hs = np.asarray(hidden_states, np.float32)
    am = np.asarray(attention_mask, np.float32)
    q_w = np.asarray(q_w, np.float32)
    k_w = np.asarray(k_w, np.float32)
    v_w = np.asarray(v_w, np.float32)
    q_b = np.asarray(q_b, np.float32)
    k_b = np.asarray(k_b, np.float32)
    v_b = np.asarray(v_b, np.float32)

    scale = np.float32(1.0 / np.sqrt(HD))

    xt_b = [np.ascontiguousarray(hs[b].T).astype(bf16) for b in range(B)]

    in_maps = []
    for c in range(NCORES):
        b = c // 4
        hg = c % 4
        cols = slice(WCOLS * hg, WCOLS * hg + WCOLS)
        mask = am[b, 0, 0, :]  # [S]
        in_maps.append(
            {
                "xt": xt_b[b],
                "wq": np.ascontiguousarray(q_w[:, cols] * scale).astype(bf16),
                "wk": np.ascontiguousarray(k_w[:, cols]).astype(bf16),
                "wv": np.ascontiguousarray(v_w[:, cols]).astype(bf16),
                "qb2": np.ascontiguousarray((q_b[cols] * scale).reshape(2, 128).T),
                "kb2": np.ascontiguousarray(k_b[cols].reshape(2, 128).T),
                "vb": np.ascontiguousarray(v_b[cols].reshape(1, WCOLS)),
                "maskc": np.ascontiguousarray(mask.reshape(16, 128).T),
            }
        )
    return in_maps


def kernel(hidden_states, attention_mask, q_w, q_b, k_w, k_b, v_w, v_b):
    from concourse import bass_utils

    nc = _get_program()
    in_maps = _make_in_maps(
        hidden_states, attention_mask, q_w, q_b, k_w, k_b, v_w, v_b
    )
    _CACHE["in_maps"] = in_maps
    res = bass_utils.run_bass_kernel_spmd(nc, in_maps, core_ids=list(range(NCORES)))

    full = np.empty((B, S, HIDDEN), np.float32)
    for c in range(NCORES):
        b = c // 4
        hg = c % 4
        full[b, :, WCOLS * hg : WCOLS * hg + WCOLS] = res.results[c]["out"].T
    return full


# revision 13
# speedup vs baseline: 1.4810x; 1.0205x over previous
"""BertSelfAttention on 8 Trainium2 NeuronCores.

Sharding: data parallel over batch (B=2) x tensor parallel over heads
(16 heads -> 4 groups of 4). Core c handles batch c//4, heads 4*(c%4)..+4.
No collectives: each core produces a disjoint [256, 2048] slice of the
output (feature-major); the host transposes/concatenates.

v3 design (baseline 286us, v2 241us):
  Jointly limited by PE (~135us of matmul) and ACT (~143-171us of exp,
  1 elem/cycle/lane, ScalarE-only); the kernel is a software pipeline
  over 8 "units" (head-pair mc x q-block sp) that keeps ACT dense:
  - inputs bf16; xt streamed as 16 half-chunk DMAs ordered so that
    K/Q projections and the first S-matmuls chase the transfer.
  - prelude K/Q projections accumulate in pq/pc PSUM slots so the
    S-ring (tag ps) is free the moment the first unit starts.
  - per unit: 16 S-pair matmuls (both heads packed via PE row groups)
    feed 16 exp ACTIVATEs; ctx matmuls of the SAME unit trail the exps
    in-step (lag-0), so there is no unit-boundary stall and no tail.
  - warmup matmuls during the DMA release the PE HAM clock gate.
  - softmax denom: reciprocal_approx_fast + gpsimd partition broadcast.
  - V bias folded into the PSUM->SBUF eviction (tensor_add).
  - expP/V in fp16 (PE rate identical to bf16/f32r for N=512).

Per-core device program:
  inputs:
    xt    [1024, 2048] bf16  hidden_states[b].T
    wq/wk/wv [1024, 256] bf16 weight column slices (wq,qb pre-scaled 1/8)
    qb2/kb2 [128, 2] f32     bias chunks (per-partition layout)
    vb    [1, 256] f32
    maskc [128, 16] f32      additive mask chunks (mask[kt*128+p] at [p, kt])
  output:
    out   [256, 2048] f32    context slice, feature-major
"""

import numpy as np

HIDDEN = 1024
HEADS = 16
HD = 64
B = 2
S = 2048
NCORES = 8
HPC = HEADS // 4  # heads per core = 4
WCOLS = HPC * HD  # 256 weight columns per core
WARM_MM = 12

_CACHE = {}


def _build_program():
    import concourse.bass as bass
    import concourse.bacc as bacc
    import concourse.tile as tile
    import concourse.mybir as mybir

    f32 = mybir.dt.float32
    f32r = mybir.dt.float32r
    bf16 = mybir.dt.bfloat16
    fp16 = mybir.dt.float16

    nc = bacc.Bacc("TRN2", target_bir_lowering=False, debug=False, num_devices=NCORES)

    xt_d = nc.dram_tensor("xt", [HIDDEN, S], bf16, kind="ExternalInput")
    wq_d = nc.dram_tensor("wq", [HIDDEN, WCOLS], bf16, kind="ExternalInput")
    wk_d = nc.dram_tensor("wk", [HIDDEN, WCOLS], bf16, kind="ExternalInput")
    wv_d = nc.dram_tensor("wv", [HIDDEN, WCOLS], bf16, kind="ExternalInput")
    qb_d = nc.dram_tensor("qb2", [128, 2], f32, kind="ExternalInput")
    kb_d = nc.dram_tensor("kb2", [128, 2], f32, kind="ExternalInput")
    vb_d = nc.dram_tensor("vb", [1, WCOLS], f32, kind="ExternalInput")
    mask_d = nc.dram_tensor("maskc", [128, 16], f32, kind="ExternalInput")
    out_d = nc.dram_tensor("out", [WCOLS, S], f32, kind="ExternalOutput")

    xt_r = xt_d.ap().rearrange("(c p) m -> p c m", p=128)  # [128, 8, 2048]
    wq_r = wq_d.ap().rearrange("(c p) n -> p c n", p=128)  # [128, 8, 256]
    wk_r = wk_d.ap().rearrange("(c p) n -> p c n", p=128)
    wv_r = wv_d.ap().rearrange("(c p) n -> p c n", p=128)

    with tile.TileContext(nc) as tc:
        with (
            tc.tile_pool(name="persist", bufs=1) as persist,
            tc.tile_pool(name="proj", bufs=1) as proj,
            tc.tile_pool(name="expp", bufs=2) as expp,
            tc.tile_pool(name="work", bufs=2) as work,
            tc.tile_pool(name="psp", bufs=1, space="PSUM") as psp,
        ):
            # ---- persistent SBUF ----
            q_sb = persist.tile([128, 2, S], f32r)  # [feat(2 heads), mc, token]
            k_sb = persist.tile([128, 2, S], f32r)
            v_sb = persist.tile([128, 16, 4 * 65], fp16)  # [tok, mt, 4*(1+64)]
            v_blk = v_sb.rearrange("p m (l c) -> p m l c", l=4)
            qkb = persist.tile([128, 20], f32)
            qb_sb = qkb[:, 0:2]
            kb_sb = qkb[:, 2:4]
            mask_sb = qkb[:, 4:20]
            vb_sb = persist.tile([1, WCOLS], f32)
            vbb = persist.tile([128, WCOLS], f32)
            vbb4 = vbb.rearrange("p (l c) -> p l c", l=4)
            wu = persist.tile([128, 512], bf16)

            # ---- input SBUF (weights + hidden states) ----
            xt = [proj.tile([128, S], bf16, tag=f"xt{k}", name=f"xt{k}") for k in range(8)]
            wq_sb = proj.tile([128, 8, WCOLS], bf16)
            wk_sb = proj.tile([128, 8, WCOLS], bf16)
            wv_sb = proj.tile([128, 8, WCOLS], bf16)

            # small inputs on the (otherwise idle) ACT queue
            nc.scalar.dma_start(out=qb_sb, in_=qb_d.ap())
            nc.scalar.dma_start(out=kb_sb, in_=kb_d.ap())
            nc.scalar.dma_start(out=mask_sb, in_=mask_d.ap())
            nc.scalar.dma_start(out=vb_sb[:], in_=vb_d.ap())
            # big stream on sync, ordered so compute chases the transfer:
            # only the mc=0 halves of wk/wq gate the prelude.
            H = S // 2
            nc.sync.dma_start(out=wk_sb[:, :, 0:128], in_=wk_r[:, :, 0:128])
            for k in range(8):
                nc.sync.dma_start(out=xt[k][:, 0:H], in_=xt_r[:, k, 0:H])
            nc.sync.dma_start(out=wq_sb[:, :, 0:128], in_=wq_r[:, :, 0:128])
            nc.sync.dma_start(out=wv_sb[:], in_=wv_r)
            for k in range(8):
                nc.sync.dma_start(out=xt[k][:, H:S], in_=xt_r[:, k, H:S])
            nc.sync.dma_start(out=wk_sb[:, :, 128:256], in_=wk_r[:, :, 128:256])
            nc.sync.dma_start(out=wq_sb[:, :, 128:256], in_=wq_r[:, :, 128:256])

            # ---- PE warmup during the input DMA (HAM clock-gate release) ----
            nc.vector.memset(wu[:], 0.0)
            warm_ps = psp.tile([128, 512], f32, tag="pq", bufs=2, name="warm")
            for i in range(WARM_MM):
                nc.tensor.matmul(
                    warm_ps[:, 0:256],
                    lhsT=wu[:, 0:128],
                    rhs=wu[:, 0:256],
                    start=True,
                    stop=True,
                )

            # ones column of V-aug (softmax denominator row source)
            nc.vector.memset(v_blk[:, :, :, 0:1], 1.0)
            # V bias broadcast to all partitions (folded into PSUM eviction)
            nc.gpsimd.partition_broadcast(vbb[:], vb_sb[0:1, :])

            # ---- building blocks ----
            def proj_group(w_sb, b_sb, dst, mc, sp, tag="pq"):
                pq = psp.tile([128, 512], f32, tag=tag, bufs=2)
                for k in range(8):
                    nc.tensor.matmul(
                        pq[:],
                        lhsT=w_sb[:, k, mc * 128 : mc * 128 + 128],
                        rhs=xt[k][:, sp * 512 : sp * 512 + 512],
                        start=(k == 0),
                        stop=(k == 7),
                    )
                nc.vector.tensor_scalar_add(
                    dst[:, mc, sp * 512 : sp * 512 + 512],
                    pq[:],
                    b_sb[:, mc : mc + 1],
                )

            def v_group(mt):
                pv = psp.tile([128, 512], f32, tag="pq", bufs=2, name=f"pv{mt}")
                for k in range(8):
                    nc.tensor.matmul(
                        pv[:, 0:WCOLS],
                        lhsT=xt[k][:, mt * 128 : mt * 128 + 128],
                        rhs=wv_sb[:, k, :],
                        start=(k == 0),
                        stop=(k == 7),
                    )
                pv4 = pv[:, 0:WCOLS].rearrange("p (l c) -> p l c", l=4)
                nc.vector.tensor_add(v_blk[:, mt, :, 1:65], pv4, vbb4[:])

            def s_unit(mc, sp, kt, expP):
                qs = sp * 512
                ps = psp.tile([128, 1024], f32, tag="ps", bufs=2)
                for half in range(2):
                    rs = 64 * half
                    nc.tensor.matmul(
                        ps[:, half * 512 : half * 512 + 512],
                        lhsT=k_sb[rs : rs + 64, mc, kt * 128 : kt * 128 + 128],
                        rhs=q_sb[rs : rs + 64, mc, qs : qs + 512],
                        start=True,
                        stop=True,
                    )
                nc.scalar.activation(
                    expP[:, kt, :],
                    ps[:],
                    mybir.ActivationFunctionType.Exp,
                    bias=mask_sb[:, kt : kt + 1],
                )

            def ctx_quanta(mc, sp, expP, half):
                """Returns 5 callables: 4 matmul chunks + normalize/store."""
                lh = 2 * mc + half
                qs = sp * 512
                pc = psp.tile([65, 512], f32, tag="pc", bufs=2, name=f"pc{lh}{sp}")

                def chunk(ci):
                    def go():
                        for kt in range(4 * ci, 4 * ci + 4):
                            nc.tensor.matmul(
                                pc[:],
                                lhsT=v_sb[:, kt, 65 * lh : 65 * lh + 65],
                                rhs=expP[:, kt, half * 512 : half * 512 + 512],
                                start=(kt == 0),
                                stop=(kt == 15),
                            )
                    return go

                def finish():
                    r = work.tile([1, 512], f32, tag="r")
                    nc.vector.reciprocal_approx_fast(r[:], pc[0:1, :])
                    bc = work.tile([65, 512], f32, tag="bc")
                    nc.gpsimd.partition_broadcast(bc[:], r[0:1, :])
                    ctxs = work.tile([65, 512], f32, tag="ctxs")
                    nc.vector.tensor_mul(ctxs[:], pc[:], bc[:])
                    nc.sync.dma_start(
                        out=out_d.ap()[64 * lh : 64 * lh + 64, qs : qs + 512],
                        in_=ctxs[1:65, :],
                    )

                return [chunk(0), chunk(1), chunk(2), chunk(3), finish]

            # ---- prelude: K(0,0), K(0,1), Q(0,0) chase the half0 stream ----
            # pq slots hold K(0,0)/K(0,1); a pc-tag slot holds Q(0,0) so the
            # S-ring (tag ps) stays free for the first unit.
            pre_k0 = psp.tile([128, 512], f32, tag="pq", bufs=2, name="preK0")
            pre_k1 = psp.tile([128, 512], f32, tag="pq", bufs=2, name="preK1")
            pre_q = psp.tile([128, 512], f32, tag="pc", bufs=2, name="preQ")
            for k in range(8):
                nc.tensor.matmul(
                    pre_k0[:],
                    lhsT=wk_sb[:, k, 0:128],
                    rhs=xt[k][:, 0:512],
                    start=(k == 0),
                    stop=(k == 7),
                )
                nc.tensor.matmul(
                    pre_k1[:],
                    lhsT=wk_sb[:, k, 0:128],
                    rhs=xt[k][:, 512:1024],
                    start=(k == 0),
                    stop=(k == 7),
                )
                nc.tensor.matmul(
                    pre_q[:],
                    lhsT=wq_sb[:, k, 0:128],
                    rhs=xt[k][:, 0:512],
                    start=(k == 0),
                    stop=(k == 7),
                )
            nc.vector.tensor_scalar_add(k_sb[:, 0, 0:512], pre_k0[:], kb_sb[:, 0:1])
            nc.vector.tensor_scalar_add(q_sb[:, 0, 0:512], pre_q[:], qb_sb[:, 0:1])
            nc.vector.tensor_scalar_add(k_sb[:, 0, 512:1024], pre_k1[:], kb_sb[:, 0:1])

            # ---- software pipeline over 8 units, ctx trails exps in-step ----
            UNITS = [(mc, sp) for mc in range(2) for sp in range(4)]
            expPs = {}
            ctx_q = {}  # u -> list of 10 quanta [h0c0..h0fin, h1c0..h1fin]

            def K(mc, sp):
                return lambda: proj_group(wk_sb, kb_sb, k_sb, mc, sp)

            def Q(mc, sp):
                return lambda: proj_group(wq_sb, qb_sb, q_sb, mc, sp)

            def fillers_for_unit(u):
                """kt-slot -> list of callables, emitted after s_unit(u, kt).

                All ctx is lag-1 (unit u runs ctx of unit u-1), except unit 7
                whose own ctx trails its exps in-step so there is no tail.
                Per-unit PE filler load is balanced against the ACT window
                (~17.8us/unit): V-projection is split across units 0-1,
                interleaved so ctx(0,0) chunk ci (which reads only V tiles
                4ci..4ci+3) finds its V tiles ready.
                """
                slots = {kt: [] for kt in range(16)}

                def put(kt, fn):
                    slots[kt].append(fn)

                def lag1(u):
                    # previous unit's ctx quanta, both heads interleaved
                    pmc, psp_ = UNITS[u - 1]
                    q = ctx_q[u - 1] = ctx_quanta(
                        pmc, psp_, expPs[u - 1], 0
                    ) + ctx_quanta(pmc, psp_, expPs[u - 1], 1)
                    return q

                if u == 0:
                    # K(0,2)/(0,3) placed late enough that their DMA-paced
                    # k-loops (half1 stream) don't head-of-line-block the
                    # weave, early enough to gate exps kt8/kt12.
                    for mt in range(5):
                        put(mt, lambda mt=mt: v_group(mt))
                    put(5, K(0, 2))
                    put(6, lambda: v_group(5))
                    put(7, K(0, 3))
                    put(8, lambda: v_group(6))
                    put(9, lambda: v_group(7))
                    put(11, Q(0, 1))
                    put(13, Q(0, 2))
                elif u == 1:
                    q = lag1(u)
                    put(0, lambda: v_group(8))
                    put(1, lambda: v_group(9))
                    put(2, q[0])                      # h0 c0
                    put(3, lambda: v_group(10))
                    put(4, q[1])                      # h0 c1
                    put(5, lambda: v_group(11))
                    put(6, q[5])                      # h1 c0
                    put(7, lambda: v_group(12))
                    put(8, q[6])                      # h1 c1
                    put(9, lambda: v_group(13))
                    put(10, q[2])                     # h0 c2
                    put(11, lambda: v_group(14))
                    put(12, q[7])                     # h1 c2
                    put(13, lambda: v_group(15))
                    put(14, q[3])                     # h0 c3
                    put(15, q[8])                     # h1 c3
                    put(15, q[4])                     # h0 finish
                    put(15, q[9])                     # h1 finish
                else:
                    q = lag1(u)
                    if u == 7:
                        # compact: free both pc slots by slot 4 so the
                        # in-step ctx(1,3) can allocate them from slot 5
                        for i, slot in zip(
                            (0, 5, 1, 6, 2, 7, 3, 8, 4, 9),
                            (0, 0, 1, 1, 2, 2, 3, 3, 4, 4),
                        ):
                            put(slot, q[i])
                    else:
                        for i, slot in zip(
                            (0, 5, 1, 6, 2, 7, 3, 8, 4, 9),
                            (0, 1, 2, 3, 4, 5, 6, 7, 8, 9),
                        ):
                            put(slot, q[i])
                    if u == 2:
                        put(10, Q(0, 3))
                        put(11, K(1, 0))
                    elif u == 3:
                        put(10, K(1, 1))
                        put(12, K(1, 2))
                        put(14, Q(1, 0))
                    elif u == 4:
                        put(0, K(1, 3))
                        put(12, Q(1, 1))
                    elif u == 5:
                        put(10, Q(1, 2))
                    elif u == 6:
                        put(10, Q(1, 3))
                    elif u == 7:
                        # own ctx trails in-step
                        mc, sp = UNITS[7]
                        qu = ctx_q[7] = ctx_quanta(
                            mc, sp, expPs[7], 0
                        ) + ctx_quanta(mc, sp, expPs[7], 1)
                        put(5, qu[0]); put(6, qu[5])
                        put(8, qu[1]); put(9, qu[6])
                        put(12, qu[2]); put(13, qu[7])
                return slots

            # Emit fillers one slot BEHIND the s-pairs: s(kt+1) directly
            # follows slot-(kt-1) fillers, so it completes during exp(kt)
            # and ACT never waits on a just-in-time semaphore.
            prev15 = []
            for u, (mc, sp) in enumerate(UNITS):
                expP = expp.tile([128, 16, 1024], fp16, tag="expP")
                expPs[u] = expP
                slots = fillers_for_unit(u)
                for kt in range(16):
                    s_unit(mc, sp, kt, expP)
                    for fn in (prev15 if kt == 0 else slots[kt - 1]):
                        fn()
                prev15 = slots[15]
            for fn in prev15:
                fn()

            # tail: last unit's c3 + finishes
            q7 = ctx_q[7]
            q7[3](); q7[8](); q7[4](); q7[9]()

    nc.compile()
    return nc


def _get_program():
    if "nc" not in _CACHE:
        _CACHE["nc"] = _build_program()
    return _CACHE["nc"]


def _make_in_maps(hidden_states, attention_mask, q_w, q_b, k_w, k_b, v_w, v_b):
    import ml_dtypes

    bf16 = ml_dtypes.bfloat16

    hs = np.asarray(hidden_states, np.float32)
    am = np.asarray(attention_mask, np.float32)
    q_w = np.asarray(q_w, np.float32)
    k_w = np.asarray(k_w, np.float32)
    v_w = np.asarray(v_w, np.float32)
    q_b = np.asarray(q_b, np.float32)
    k_b = np.asarray(k_b, np.float32)
    v_b = np.asarray(v_b, np.float32)

    scale = np.float32(1.0 / np.sqrt(HD))

    xt_b = [np.ascontiguousarray(hs[b].T).astype(bf16) for b in range(B)]

    in_maps = []
    for c in range(NCORES):
        b = c // 4
        hg = c % 4
        cols = slice(WCOLS * hg, WCOLS * hg + WCOLS)
        mask = am[b, 0, 0, :]  # [S]
        in_maps.append(
            {
                "xt": xt_b[b],
                "wq": np.ascontiguousarray(q_w[:, cols] * scale).astype(bf16),
                "wk": np.ascontiguousarray(k_w[:, cols]).astype(bf16),
                "wv": np.ascontiguousarray(v_w[:, cols]).astype(bf16),
                "qb2": np.ascontiguousarray((q_b[cols] * scale).reshape(2, 128).T),
                "kb2": np.ascontiguousarray(k_b[cols].reshape(2, 128).T),
                "vb": np.ascontiguousarray(v_b[cols].reshape(1, WCOLS)),
                "maskc": np.ascontiguousarray(mask.reshape(16, 128).T),
            }
        )
    return in_maps


def kernel(hidden_states, attention_mask, q_w, q_b, k_w, k_b, v_w, v_b):
    from concourse import bass_utils

    nc = _get_program()
    in_maps = _make_in_maps(
        hidden_states, attention_mask, q_w, q_b, k_w, k_b, v_w, v_b
    )
    _CACHE["in_maps"] = in_maps
    res = bass_utils.run_bass_kernel_spmd(nc, in_maps, core_ids=list(range(NCORES)))

    full = np.empty((B, S, HIDDEN), np.float32)
    for c in range(NCORES):
        b = c // 4
        hg = c % 4
        full[b, :, WCOLS * hg : WCOLS * hg + WCOLS] = res.results[c]["out"].T
    return full
